# revision 30
# baseline (speedup 1.0000x reference)
"""SigLIP2 attention block on 8 TRN2 NeuronCores.

Strategy: data-parallel over batch (B=8 -> 1 batch element per core, no
collectives). Weights/activations pre-quantized to fp8(e4m3) hi/lo pairs on
the host so QKV runs as DoubleRow fp8 matmuls (2 K-tiles per instruction at
0.5 cycles/row = 4x bf16 MAC throughput) with a 3-pass hi/lo compensation
scheme (hi*hi k-tile pairs; per-tile (hi*lo + lo*hi) packed into single
DoubleRow instructions) keeping quantization error ~1e-3.

Layout trick: q/k projection rows are host-permuted into 128-row tiles of
[64 first-half-hd rows | 64 second-half-hd rows] (seqA/seqB interleave), so
the rotate-half partner of every row lives exactly 64 partitions away in the
SAME tile. Rope then runs at full 128-partition DVE utilization with legal
64-aligned partition-shifted multiplies and no cross-tile dependencies:

  per core (batch b):
    qkv:    q/k psum[j,s] = sum_t Wt[j].T @ Xt[s]  via 5 main-DR + 9 corr-DR
            v   psum[s,j] = sum_t Xt[s].T @ Vt[j]  same 14-instruction scheme
    rope:   qraw <- psum * dq (ACT copy); qk8[j] = qraw*cos_rep +
            shift64(qraw)*sin_rep -> fp8 (3 DVE muls + 1 add per tile);
            cos_rep/sin_rep[r] = cos/sin[hd(r)] host-packed, streamed.
    attn:   scores_T[ks,qs] = DoubleRow fp8 over hd (36+36 halves) reading
            per-head gathers qh8/kh8[36, 2x1024] (gpsimd DMA, issued one
            schedule step ahead); exp on ACT (scale=1/sqrt(72), no
            max-subtract: |scores| is O(1)); PV: attn_T[hd,qs] +=
            vpad[ks,97].T @ exp[ks,qs] with a ones column at 96 so the
            softmax denominator lands at psum partition 96. Broadcast denom
            via K=1 ones matmul, reciprocal+normalize on DVE, DMA pack
            into attn_T[f,s].
    proj:   bf16, out[s,e] = attn_packed[f,s].T @ proj_wT[f,e]
  proj_b added on host (linear); qkv_b is all-zero in this problem (asserted).

Scheduling: fine-grained emission interleaving — each head's 8 scores tiles
cost PE only ~214ns but gate on ~1us exp (2-buf psum recycling), so scores
units are riffled 2:1 with QKV/V/PV matmul units; rope DVE ops are split
into 4 thunks riffled between PV posts; per-head gathers are issued one
schedule step ahead of their attention, split across the gpsimd SWDGE and
SP HWDGE queues.
"""

import os
import sys
import numpy as np

sys.path.insert(0, "/opt/trn_rl_repo")

B, S, D = 8, 1024, 1152
H, HD = 16, 72
HHD = HD // 2  # 36
NQK = 2 * D    # 2304 q+k rows
P = 128
NCORES = 8
SCALE = float(HD) ** -0.5

S_X = 16.0     # fp8 scale for hidden states
S_W = 1024.0   # fp8 scale for qkv weights
DQ = 1.0 / (S_X * S_W)
USE_FP8_SCORES = True

_CACHE = {}


def _qk_perm():
    """Permuted q/k row order: per group (q, k), 9 tiles of 128 rows =
    [seqA[64k:64k+64] | seqB[64k:64k+64]] where seqA = all heads' hd 0:36
    and seqB = all heads' hd 36:72. Returns perm[2304] -> original row."""
    perm = np.zeros(NQK, np.int64)
    for g in range(2):          # q, k
        for kt in range(9):
            for p in range(P):
                if p < 64:
                    pos = 64 * kt + p
                    h, hd = pos // HHD, pos % HHD
                else:
                    pos = 64 * kt + (p - 64)
                    h, hd = pos // HHD, HHD + pos % HHD
                perm[g * 1152 + kt * P + p] = g * 1152 + h * HD + hd
    return perm


def _build(reps=1):
    import concourse.bass as bass
    import concourse.bacc as bacc
    import concourse.mybir as mybir
    from concourse import tile

    bf16 = mybir.dt.bfloat16
    f8 = mybir.dt.float8e4
    f32 = mybir.dt.float32
    DR = mybir.MatmulPerfMode.DoubleRow
    Exp = mybir.ActivationFunctionType.Exp
    qk_dt = f8 if USE_FP8_SCORES else bf16

    nc = bacc.Bacc(None)

    # hT_all: 10 blocks of 2048 cols: [t][lo(1024) | hi(1024)], block 9 zero
    hT_d = nc.declare_dram_parameter("hT", [P, 10 * 2048], f8, isOutput=False)
    # rep: [9 tiles][128, 2048]: cols 0:1024 = cos_rep, 1024:2048 = sin_rep
    rep_d = nc.declare_dram_parameter("repCS", [9 * P, 2048], bf16,
                                      isOutput=False)
    # qkw: per j-tile [128, 10*256]: blocks [t][hi(128) | lo(128)], block9 = 0
    qkwT_d = nc.declare_dram_parameter("qkwT", [18 * P, 2560], f8,
                                       isOutput=False)
    # vw: [128, 10*2304]: blocks [t][hi(1152) | lo(1152)], block9 = 0
    vwT_d = nc.declare_dram_parameter("vwT", [P, 23040], f8, isOutput=False)
    pwT_d = nc.declare_dram_parameter("pwT", [D, D], bf16, isOutput=False)
    ident_d = nc.declare_dram_parameter("ident", [P, P], bf16,
                                        isOutput=False)
    out_d = nc.declare_dram_parameter("out", [S, D], f32, isOutput=True)

    ND = D // P      # 9 d tiles
    NS = S // P      # 8 s tiles
    VP = 73          # head dim + denominator column at 72 (free dim)
    VPADW = H * VP   # 1168

    # head h's q/k rows live at seq positions h*36..h*36+36 in both the A
    # and B 64-row blocks -> last tile = (h*36+35)//64
    jq = [(h * HHD + HHD - 1) // 64 for h in range(H)]
    gstep = {}   # step -> heads to gather (after this step's ropes)
    astep = {}   # step -> heads to run attention
    for h in range(H):
        gstep.setdefault(jq[h], []).append(h)
        astep.setdefault(jq[h] + 1, []).append(h)
    NSTEP = ND + 1  # 10
    # attn_T f-tile j is transposable once its last covering head is done
    tr_after = {}
    for j in range(ND):
        last_h = ((j + 1) * P + HD - 1) // HD - 1
        tr_after.setdefault(min(last_h, H - 1), []).append(j)

    with tile.TileContext(nc) as tc:
        with (
            tc.tile_pool(name="persist", bufs=1) as pp,
            tc.tile_pool(name="work", bufs=2) as wp,
            tc.tile_pool(name="gat", bufs=4) as gp,
            tc.tile_pool(name="expp", bufs=10) as ep,
            tc.tile_pool(name="psp", bufs=2, space="PSUM") as psp,
        ):
            def _emit_once():
                hT = pp.tile([P, 10 * 2048], f8, tag="hT", name="hT")
                vwT = pp.tile([P, 23040], f8, tag="vwT", name="vwT")
                ident = pp.tile([P, P], bf16, tag="ident", name="ident")
                stg = [pp.tile([P, D], bf16, tag=f"stg{i}", name=f"stg{i}")
                       for i in range(NS)]
                qk8 = [pp.tile([P, S], qk_dt, tag=f"qk8_{j}",
                               name=f"qk8_{j}") for j in range(18)]
                vpad = [pp.tile([P, VPADW], bf16, tag=f"vp{i}",
                                name=f"vp{i}") for i in range(NS)]
                attnp = [pp.tile([P, S], bf16, tag=f"at{i}",
                                 name=f"at{i}") for i in range(ND)]

                # split the big resident loads across SP and ACT HWDGE queues
                nc.sync.dma_start(hT[:, 0:10240], hT_d[:, 0:10240])
                nc.scalar.dma_start(hT[:, 10240:20480], hT_d[:, 10240:20480])
                nc.scalar.dma_start(vwT[:], vwT_d[:, :])
                nc.sync.dma_start(ident[:], ident_d[:, :])

                hT2k = hT[:].rearrange("p (t c) -> p t c", c=2048)
                hT1k = hT[:].rearrange("p (t c) -> p t c", c=1024)
                vw2 = vwT[:].rearrange("p (t c) -> p t c", c=2304)
                vw1 = vwT[:].rearrange("p (t c) -> p t c", c=1152)

                def qkv_mms(ps, stat_of, mov_of):
                    """14 DR matmuls accumulating the 3-pass hi/lo scheme."""
                    for i in range(5):      # main: (hi_2i, hi_2i+1)
                        nc.tensor.matmul(ps, stat_of("main", i),
                                         mov_of("main", i), start=(i == 0),
                                         stop=False, perf_mode=DR)
                    for t in range(ND):     # corr: Xl*Wh + Xh*Wl
                        nc.tensor.matmul(ps, stat_of("corr", t),
                                         mov_of("corr", t), start=False,
                                         stop=(t == ND - 1), perf_mode=DR)

                # ---- v: [s, j] orientation, per-head 97 cols + ones col ----
                def v_unit(st, hc):
                    """One v chunk: 4 heads x 72 cols for s-tile st."""
                    if hc == 0:
                        nc.vector.memset(vpad[st][:], 1.0)
                    ps = psp.tile([P, 512], f32, tag="small", bufs=2,
                                  name="vps")

                    def stat_of(kind, i, st=st):
                        if kind == "main":
                            return hT2k[:, 2 * i:2 * i + 2,
                                        1024 + st * P:1024 + st * P + P]
                        return hT1k[:, 2 * i:2 * i + 2, st * P:st * P + P]

                    def mov_of(kind, i, hc=hc):
                        c0 = hc * 288
                        if kind == "main":
                            return vw2[:, 2 * i:2 * i + 2, c0:c0 + 288]
                        return vw1[:, 2 * i:2 * i + 2, c0:c0 + 288]

                    qkv_mms(ps[:, 0:288], stat_of, mov_of)
                    dst = vpad[st][:].rearrange(
                        "p (h c) -> p h c", c=VP)[:, hc * 4:(hc + 1) * 4,
                                                  0:HD]
                    nc.vector.tensor_scalar_mul(dst, ps[:, 0:288], DQ)

                def qk_tile_units(jt, wsp, qrp):
                    """Issue the weight DMA now; return (qraw, [unit0,
                    unit1]) where each unit emits one 512-col psum chunk."""
                    w = wsp.tile([P, 2560], f8, tag="wjt", name="wjt")
                    nc.sync.dma_start(w[:], qkwT_d[jt * P:(jt + 1) * P, :])
                    w256 = w[:].rearrange("p (t c) -> p t c", c=256)
                    w128 = w[:].rearrange("p (t c) -> p t c", c=128)
                    qraw = qrp.tile([P, S], bf16, tag="qraw",
                                    name=f"qraw{jt}")

                    def unit(sc):
                        def f():
                            ps = psp.tile([P, 512], f32, tag="small", bufs=2,
                                          name="qkps")
                            s0 = sc * 512

                            def stat_of(kind, i):
                                if kind == "main":
                                    return w256[:, 2 * i:2 * i + 2, 0:P]
                                return w128[:, 2 * i:2 * i + 2, 0:P]

                            def mov_of(kind, i):
                                if kind == "main":
                                    return hT2k[:, 2 * i:2 * i + 2,
                                                1024 + s0:1024 + s0 + 512]
                                return hT1k[:, 2 * i:2 * i + 2,
                                            s0:s0 + 512]

                            qkv_mms(ps[:], stat_of, mov_of)
                            nc.vector.tensor_scalar_mul(
                                qraw[:, s0:s0 + 512], ps[:], DQ)
                        return f

                    return qraw, [unit(0), unit(1)]

                def rope_units(jt, qraw, rep):
                    """qk8[jt] = qraw*cos_rep + shift64(qraw)*sin_swap, as 4
                    separate DVE thunks so they can be riffled between the
                    PV posts (keeps the in-order DVE queue from blocking
                    PE's denominator matmuls). Partner rows are exactly 64
                    partitions away (layout); sin_swap is source-indexed so
                    each multiply's SBUF inputs share a base partition."""
                    cr, sw = rep[:, 0:S], rep[:, S:2 * S]
                    rot = wp.tile([P, S], bf16, tag="rot", name="rot")
                    qcos = wp.tile([P, S], bf16, tag="qcos", name="qcos")
                    return [
                        lambda: nc.vector.tensor_mul(qcos[:], qraw[:],
                                                     cr[:]),
                        lambda: nc.vector.tensor_mul(
                            rot[0:64, :], qraw[64:P, :], sw[64:P, :]),
                        lambda: nc.vector.tensor_mul(
                            rot[64:P, :], qraw[0:64, :], sw[0:64, :]),
                        lambda: nc.vector.tensor_add(qk8[jt][:], qcos[:],
                                                     rot[:]),
                    ]

                def seg_gather(dst, col0, g, h, half, eng):
                    """dst[0:36, col0:col0+S] <- head h's q/k (g) rows for
                    hd-half `half` from the permuted qk8 tiles."""
                    pos, dst_row, n = h * HHD, 0, HHD
                    while n > 0:
                        kt, r = pos // 64, pos % 64
                        c = min(n, 64 - r)
                        src = qk8[g * ND + kt]
                        r0 = r + (64 if half else 0)
                        eng.dma_start(
                            dst[dst_row:dst_row + c, col0:col0 + S],
                            src[r0:r0 + c, :])
                        dst_row += c
                        pos += c
                        n -= c

                gath = {}

                def emit_gather(h):
                    # q gathers on the gpsimd SWDGE queue, k gathers on the
                    # (otherwise idle) SP HWDGE queue to halve Pool load
                    qh = gp.tile([HHD, 2 * S], f8, tag="qh", name="qh")
                    kh = gp.tile([HHD, 2 * S], f8, tag="kh", name="kh")
                    seg_gather(qh, 0, 0, h, 0, nc.gpsimd)
                    seg_gather(qh, S, 0, h, 1, nc.gpsimd)
                    seg_gather(kh, 0, 1, h, 0, nc.sync)
                    seg_gather(kh, S, 1, h, 1, nc.sync)
                    gath[h] = (qh, kh)

                def scores_units(h):
                    """Returns (ex_list, [unit per kt-PAIR]); each unit
                    emits 4 DR matmuls filling a 4-bank psum + ONE exp of
                    [128, 2048] -- halves the ACT per-instruction init
                    overhead (attention steps are ACT-bound)."""
                    qh, kh = gath[h]
                    qh2 = qh[:].rearrange("p (two n) -> p two n", two=2)
                    kh2 = kh[:].rearrange("p (two n) -> p two n", two=2)
                    ex = []

                    def unit(pair):
                        def f():
                            e = ep.tile([P, 2 * S], bf16, tag="exp",
                                        name="exp", bufs=10)
                            ps = psp.tile([P, 2 * S], f32, tag="big",
                                          bufs=1, name="sps")
                            for half in range(2):
                                kt = 2 * pair + half
                                for qc in range(2):
                                    c0 = half * S + qc * 512
                                    nc.tensor.matmul(
                                        ps[:, c0:c0 + 512],
                                        kh2[:, :, kt * P:(kt + 1) * P],
                                        qh2[:, :, qc * 512:(qc + 1) * 512],
                                        start=True, stop=True, perf_mode=DR)
                            nc.scalar.activation(e[:], ps[:], Exp,
                                                 scale=SCALE)
                            ex.append(e)
                        return f

                    return ex, [unit(pr) for pr in range(4)]

                def pv_unit(h, ex, qt):
                    """PV for one (head, q-tile): out psum [qs=128, 73] with
                    the softmax denominator in free column 72 (ones column
                    of vpad), so cost is 73 cycles/matmul and the normalize
                    is a per-partition tensor_scalar into staging[qt]."""
                    def f():
                        ps = psp.tile([P, 512], f32, tag="apvt", bufs=2,
                                      name="apv")
                        for kt in range(NS):
                            c0 = (kt % 2) * S + qt * P
                            nc.tensor.matmul(
                                ps[:, 0:VP],
                                ex[kt // 2][:, c0:c0 + P],
                                vpad[kt][:, h * VP:(h + 1) * VP],
                                start=(kt == 0), stop=(kt == NS - 1))
                        rb1 = wp.tile([P, 1], f32, tag="rb1", name="rb1",
                                      bufs=3)
                        nc.vector.reciprocal(rb1[:], ps[:, HD:HD + 1])
                        nc.vector.tensor_scalar_mul(
                            stg[qt][:, h * HD:(h + 1) * HD],
                            ps[:, 0:HD], rb1[:])
                    return f

                def transpose_unit(j):
                    """attnp[j][f, s] <- transpose of staging[:, j-cols]."""
                    def f():
                        pt = psp.tile([P, 1024], bf16, tag="apvt",
                                      bufs=2, name="ptr")
                        for qt in range(NS):
                            nc.tensor.matmul(
                                pt[:, qt * P:(qt + 1) * P],
                                stg[qt][:, j * P:(j + 1) * P],
                                ident[:], is_transpose=True,
                                start=(qt == 0), stop=(qt == NS - 1))
                        nc.vector.tensor_copy(attnp[j][:], pt[:])
                    return f

                # ---------------- schedule ----------------
                # Fine-grained emission interleaving: a head's 8 scores
                # units each cost PE only ~214ns but gate on ~1us exp
                # (2-buf psum recycling), so alternate [2 scores][1 matmul
                # unit] to keep PE fed; PV units of the first head pad the
                # second head's scores.
                with (
                    tc.tile_pool(name="wstream", bufs=3) as wsp,
                    tc.tile_pool(name="qraw", bufs=3) as qrp,
                    tc.tile_pool(name="rep", bufs=2) as repp,
                ):
                    vq = [(st, hc) for st in range(NS) for hc in range(4)]
                    carry = []   # deferred PV/transpose units from prev step
                    for i in range(NSTEP):
                        ah = astep.get(i, [])
                        ropes = []
                        filler = list(carry)
                        carry = []
                        if i < ND:
                            rep = repp.tile([P, 2048], bf16, tag="rep",
                                            name=f"rep{i}")
                            nc.sync.dma_start(
                                rep[:], rep_d[i * P:(i + 1) * P, :])
                            qa, ua = qk_tile_units(i, wsp, qrp)
                            qb, ub = qk_tile_units(ND + i, wsp, qrp)
                            filler += ua + ub
                            ropes = [(i, qa, rep), (ND + i, qb, rep)]
                        if i < 2:
                            nv = 16 if i == 0 else len(vq)
                            filler += [
                                (lambda st=st, hc=hc: v_unit(st, hc))
                                for st, hc in vq[:nv]]
                            vq = vq[nv:]

                        s1 = s2 = None
                        if ah:
                            s1 = scores_units(ah[0])
                        if len(ah) > 1:
                            s2 = scores_units(ah[1])

                        def drain(units, n):
                            for _ in range(n):
                                if units:
                                    units.pop(0)()

                        # phase 1: head-1 scores interleaved with filler
                        if s1 is not None:
                            su = list(s1[1])
                            while su:
                                drain(su, 1)
                                drain(filler, 1)
                        drain(filler, len(filler))
                        # phase 2: head-2 scores interleaved with head-1
                        # PV q-tiles and the rope ops (riffled so PV's DVE
                        # posts never queue behind a full rope block)
                        aux = []
                        for jt, qr, rp in ropes:
                            aux += rope_units(jt, qr, rp)
                        pv1 = ([pv_unit(ah[0], s1[0], qt)
                                for qt in range(NS)] if ah else [])
                        mix = []
                        for a, b in zip(pv1, aux):
                            mix += [a, b]
                        n = min(len(pv1), len(aux))
                        mix += pv1[n:] + aux[n:]
                        if s2 is not None:
                            su = list(s2[1])
                            while su:
                                drain(su, 1)
                                drain(mix, 4)
                        drain(mix, len(mix))
                        for h in gstep.get(i, []):
                            emit_gather(h)
                        if ah:
                            for j in tr_after.get(ah[0], []):
                                transpose_unit(j)()
                        if s2 is not None:
                            # defer head-2 PV + its transposes into the next
                            # step's phase-1 filler so they riffle with the
                            # next head's scores instead of draining dry
                            carry += [pv_unit(ah[1], s2[0], qt)
                                      for qt in range(NS)]
                            carry += [transpose_unit(j)
                                      for j in tr_after.get(ah[1], [])]
                    for u in carry:
                        u()

                # ---- proj (bf16) ----
                with tc.tile_pool(name="projw", bufs=1) as pwp:
                    pwT = [pwp.tile([P, D], bf16, tag=f"pwT{i}",
                                    name=f"pwT{i}") for i in range(ND)]
                    for i in range(ND):
                        nc.sync.dma_start(pwT[i][:],
                                          pwT_d[i * P:(i + 1) * P, :])
                    for st in range(NS):
                        for ec in range(3):
                            ps = psp.tile([P, 512], f32, tag="small", bufs=2,
                                          name="ops")
                            for ft in range(ND):
                                nc.tensor.matmul(
                                    ps[:, 0:384],
                                    attnp[ft][:, st * P:(st + 1) * P],
                                    pwT[ft][:, ec * 384:(ec + 1) * 384],
                                    start=(ft == 0), stop=(ft == ND - 1))
                            osb = wp.tile([P, 384], f32, tag="osb",
                                          name="osb")
                            nc.scalar.copy(osb[:], ps[:, 0:384])
                            nc.sync.dma_start(
                                out_d[st * P:(st + 1) * P,
                                      ec * 384:(ec + 1) * 384], osb[:])

            for _rep in range(reps):
                _emit_once()

    nc.compile()
    return nc


def _get_nc():
    if "nc" not in _CACHE:
        _CACHE["nc"] = _build()
    return _CACHE["nc"]


def _hilo(x, s):
    """fp8 e4m3 hi/lo split of x*s. Returns (hi, lo) as fp8 arrays."""
    import ml_dtypes

    f8 = ml_dtypes.float8_e4m3
    xs = np.asarray(x, np.float32) * s
    hi = xs.astype(f8)
    lo = (xs - hi.astype(np.float32)).astype(f8)
    return hi, lo


def prep_in_maps(hidden_states, cos, sin, qkv_w, qkv_b, proj_w, proj_b):
    import ml_dtypes

    bf = ml_dtypes.bfloat16
    f8 = ml_dtypes.float8_e4m3
    hidden_states = np.asarray(hidden_states, dtype=np.float32)
    cos = np.asarray(cos, dtype=np.float32)
    sin = np.asarray(sin, dtype=np.float32)
    qkv_w = np.asarray(qkv_w, dtype=np.float32)
    qkv_b = np.asarray(qkv_b, dtype=np.float32)
    proj_w = np.asarray(proj_w, dtype=np.float32)

    assert np.abs(qkv_b).max() == 0.0, "nonzero qkv_b not supported"

    perm = _qk_perm()

    # rep tiles [9, 128, 2048]: per tile k, row p: hd(p) = (64k+p%64) % 36
    # (+36 for p>=64). cols 0:1024 = cos_rep[p] = cos[:, hd(p)] (dst-indexed)
    # cols 1024:2048 = sin_swap[p] = signed sin for the PARTNER row p^64
    # (source-indexed): row p feeds rot[p^64] = qraw[p] * sin_swap[p], where
    # rot sign is negative at first-half (p^64 < 64) destinations.
    rep_pack = np.zeros((9, P, 2048), np.float32)
    for kt in range(9):
        for p in range(P):
            if p < 64:
                hd = (64 * kt + p) % HHD
                rep_pack[kt, p, 0:1024] = cos[:, hd]
                # partner dst row p+64 has hd+36, sign +
                rep_pack[kt, p, 1024:2048] = sin[:, hd + HHD]
            else:
                hd = HHD + (64 * kt + (p - 64)) % HHD
                rep_pack[kt, p, 0:1024] = cos[:, hd]
                # partner dst row p-64 has hd-36, sign -
                rep_pack[kt, p, 1024:2048] = -sin[:, hd - HHD]
    rep_pack = rep_pack.reshape(9 * P, 2048).astype(bf)

    # qkw: [18*128, 2560] with host row-permutation applied
    qkwT = np.ascontiguousarray(qkv_w[:NQK][perm].T)              # [1152,2304]
    wh, wl = _hilo(qkwT, S_W)
    qkw_pack = np.zeros((18, P, 2560), f8)
    for j in range(18):
        for t in range(9):
            qkw_pack[j, :, t * 256:t * 256 + 128] = \
                wh[t * P:(t + 1) * P, j * P:(j + 1) * P]
            qkw_pack[j, :, t * 256 + 128:t * 256 + 256] = \
                wl[t * P:(t + 1) * P, j * P:(j + 1) * P]
    qkw_pack = qkw_pack.reshape(18 * P, 2560)

    # vw: [128, 10*2304]: blocks [hi(1152) | lo(1152)], blk9 = 0
    vwT = np.ascontiguousarray(qkv_w[NQK:].T)                     # [1152,1152]
    vh, vl = _hilo(vwT, S_W)
    vw_pack = np.zeros((P, 23040), f8)
    for t in range(9):
        vw_pack[:, t * 2304:t * 2304 + 1152] = vh[t * P:(t + 1) * P, :]
        vw_pack[:, t * 2304 + 1152:t * 2304 + 2304] = vl[t * P:(t + 1) * P, :]

    pwT = np.ascontiguousarray(proj_w.T).astype(bf)               # [1152,1152]

    in_maps = []
    for b in range(NCORES):
        hTb = np.ascontiguousarray(hidden_states[b].T)            # [1152,1024]
        xh, xl = _hilo(hTb, S_X)
        hT_pack = np.zeros((P, 10 * 2048), f8)
        for t in range(9):
            hT_pack[:, t * 2048:t * 2048 + 1024] = xl[t * P:(t + 1) * P, :]
            hT_pack[:, t * 2048 + 1024:t * 2048 + 2048] = \
                xh[t * P:(t + 1) * P, :]
        in_maps.append({
            "hT": hT_pack,
            "repCS": rep_pack,
            "qkwT": qkw_pack, "vwT": vw_pack, "pwT": pwT,
            "ident": np.eye(P, dtype=bf),
        })

    return in_maps


def kernel(hidden_states, cos, sin, qkv_w, qkv_b, proj_w, proj_b,
           _profile=False):
    from concourse.bass_utils import run_bass_kernel_spmd

    proj_b = np.asarray(proj_b, dtype=np.float32)
    in_maps = prep_in_maps(hidden_states, cos, sin, qkv_w, qkv_b,
                           proj_w, proj_b)
    nc = _get_nc()
    res = run_bass_kernel_spmd(nc, in_maps, core_ids=list(range(NCORES)),
                               trace=_profile)
    _CACHE["last_exec_time_ns"] = res.exec_time_ns
    out = np.stack([np.asarray(res.results[b]["out"], dtype=np.float32)
                    for b in range(NCORES)])
    return out + proj_b[None, None, :]
